# revision 18
# baseline (speedup 1.0000x reference)
"""Trainium2 Bass kernel for a bidirectional cross-attention block.

Reference computation (per batch b):
  t = LN(text[b]);  v = LN(vision[b])
  text_out[b]   = softmax((t@Wq1.T+bq1) (v@Wk2.T+bk2)^T / 8) (v@Wv2.T+bv2) @ Wo1.T + bo1
  vision_out[b] = softmax((v@Wq2.T+bq2) (t@Wk1.T+bk1)^T / 8) (t@Wv1.T+bv1) @ Wo2.T + bo2
  (12 heads of dk=64; D=768, N=2048)

Sharding over 8 cores: (batch b in {0,1}) x (path in {text-q, vision-q}) x
(head-half in {heads 0-5, heads 6-11}).  Each core computes a [2048, 768]
partial of one output (its 6 heads pushed through the output projection);
the host sums the two head-half partials and adds the output bias.

Device kernel (per core) highlights:
  - Host passes x TRANSPOSED (feature-major, bf16) so every matmul contracts
    along partitions; activations are never transposed on device.
  - LN scale/shift folded into the projection weights host-side (exact).
    LN mean/rstd computed on device:  Q = r*(x@W) - (r*mu) x colsum(W),
    applied as two DVE passes per projection tile (colsum identity).
  - K-side biases are row-constant in the scores -> dropped (softmax
    invariant).  The Q bias enters through a per-key correction t_k =
    bq . K_k, computed with tiny matmuls and folded into the exp() bias.
  - exp on ACT with fused scale 1/8 + bias; no max-subtraction (scores are
    bounded by ~6 for LN'd inputs).
  - Row-sums of exp(S) obtained by appending a ones-column to V in the
    P@V matmul; normalization applied after P@V (linearity).
"""

import math
import os
import sys
from contextlib import ExitStack

import numpy as np

for _p in ("/opt/trn_rl_repo", os.path.expanduser("~/.axon_site/_ro/trn_rl_repo")):
    if os.path.isdir(_p) and _p not in sys.path:
        sys.path.insert(0, _p)

import ml_dtypes  # noqa: E402

import concourse.bass as bass  # noqa: E402
import concourse.bacc as bacc  # noqa: E402
import concourse.tile as tile  # noqa: E402
from concourse import mybir  # noqa: E402
from concourse.bass_utils import run_bass_kernel_spmd  # noqa: E402

BF16 = np.dtype(np.float16)  # fp16: same PE rate as bf16, 8x mantissa

NSEQ = 2048
D = 768
HEADS = 12
DK = 64
HPC = 6            # heads per core
F = HPC * DK       # 384 features per core
KT = D // 128      # 6 contraction tiles
FB = F // 128      # 3 feature blocks
NB4 = NSEQ // 512  # 4 seq blocks of 512
NB16 = NSEQ // 128  # 16 seq blocks of 128
EPS = 1e-5
SCALE = DK ** -0.5  # 0.125

_AF = None  # mybir.ActivationFunctionType alias, set in _build


def _bcast_ap(ap, p):
    """[1, ...] SBUF/DRAM AP -> partition-broadcast [p, ...] AP (stride 0)."""
    return bass.AP(tensor=ap.tensor, offset=ap.offset, ap=[[0, p]] + list(ap.ap[1:]))


def _emit(ctx, tc, io):
    nc = tc.nc
    f32 = mybir.dt.float32
    bf16 = mybir.dt.float16
    AF = mybir.ActivationFunctionType
    OP = mybir.AluOpType

    xqT, xkvT, xkv_sm = io["xqT"], io["xkvT"], io["xkv_sm"]
    wq, wk, wv, wo = io["wq"], io["wk"], io["wv"], io["wo"]
    cnq, cnk, cnv, bvr, bqc = io["cnq"], io["cnk"], io["cnv"], io["bvr"], io["bqc"]
    out = io["out"]

    # ---- pools that live for the whole kernel ----
    const = ctx.enter_context(tc.tile_pool(name="const", bufs=1))
    qkv_pool = ctx.enter_context(tc.tile_pool(name="qkv", bufs=1))
    wo_pool = ctx.enter_context(tc.tile_pool(name="wo", bufs=1))
    tpool = ctx.enter_context(tc.tile_pool(name="tsb", bufs=1))
    apool = ctx.enter_context(tc.tile_pool(name="afm", bufs=1))

    ones = const.tile([128, 1], bf16)
    nc.vector.memset(ones, 1.0)
    eps1 = const.tile([1, 1], f32)
    nc.vector.memset(eps1, EPS)
    eps128 = const.tile([128, 1], f32)
    nc.vector.memset(eps128, EPS)
    cnq_sb = const.tile([128, FB], f32)
    nc.sync.dma_start(out=cnq_sb, in_=cnq)
    cnk_sb = const.tile([128, FB], f32)
    nc.sync.dma_start(out=cnk_sb, in_=cnk)
    bqc_sb = const.tile([128, FB], bf16)
    nc.sync.dma_start(out=bqc_sb, in_=bqc)
    cnv_bc = const.tile([128, F], f32)
    nc.gpsimd.dma_start(out=cnv_bc, in_=_bcast_ap(cnv[None, :], 128))
    bv_bc = const.tile([128, F], f32)
    nc.gpsimd.dma_start(out=bv_bc, in_=_bcast_ap(bvr[None, :], 128))

    qt_sb = qkv_pool.tile([128, FB, NSEQ], bf16)   # Q^T feature-major
    kt_sb = qkv_pool.tile([128, FB, NSEQ], bf16)   # K^T feature-major
    v_sb = qkv_pool.tile([128, NB16, HPC, DK + 1], bf16)  # V seq-major + ones col
    a_sb = apool.tile([128, FB, NSEQ], bf16)       # attention out, feature-major

    wo_sb = wo_pool.tile([128, FB, D], bf16)
    for f3 in range(FB):
        nc.sync.dma_start(out=wo_sb[:, f3, :], in_=wo[f3])

    t_sb = tpool.tile([128, HPC, NB16], f32)       # per-key exp-bias (q-bias fold)

    # ================= phase A: load x, stats, projections =================
    with ExitStack() as pre:
        xpool = pre.enter_context(tc.tile_pool(name="xt", bufs=1))
        wpool = pre.enter_context(tc.tile_pool(name="wqkv", bufs=1))
        spool = pre.enter_context(tc.tile_pool(name="stats", bufs=1))
        scr = pre.enter_context(tc.tile_pool(name="scratch", bufs=2))
        sqpool = pre.enter_context(tc.tile_pool(name="sq", bufs=2))
        bcpool = pre.enter_context(tc.tile_pool(name="bc", bufs=1))
        ppool = pre.enter_context(tc.tile_pool(name="pp", bufs=1))
        xsm_pool = pre.enter_context(tc.tile_pool(name="xsm", bufs=3))
        bnp = pre.enter_context(tc.tile_pool(name="bn", bufs=3))
        upool = pre.enter_context(tc.tile_pool(name="u", bufs=3))
        pst = pre.enter_context(tc.tile_pool(name="pstat", bufs=1, space="PSUM"))
        prj_ps = pre.enter_context(tc.tile_pool(name="prj", bufs=1, space="PSUM"))

        xqT_sb = xpool.tile([128, KT, NSEQ], bf16)
        xkvT_sb = xpool.tile([128, KT, NSEQ], bf16)
        for kt in range(KT):
            nc.sync.dma_start(out=xqT_sb[:, kt, :], in_=xqT[kt * 128:(kt + 1) * 128, :])
            nc.sync.dma_start(out=xkvT_sb[:, kt, :], in_=xkvT[kt * 128:(kt + 1) * 128, :])

        wq_sb = wpool.tile([128, KT, F], bf16)
        wk_sb = wpool.tile([128, KT, F], bf16)
        wv_sb = wpool.tile([128, KT, F], bf16)
        for kt in range(KT):
            nc.sync.dma_start(out=wq_sb[:, kt, :], in_=wq[kt])
            nc.sync.dma_start(out=wk_sb[:, kt, :], in_=wk[kt])
            nc.sync.dma_start(out=wv_sb[:, kt, :], in_=wv[kt])

        # ---- row stats (mean, rstd along features) via ones-matmuls ----
        bcs = {}
        for nm, xsb in (("q", xqT_sb), ("kv", xkvT_sb)):
            mu_bf = spool.tile([1, NSEQ], bf16, name=f"mu_bf_{nm}")
            r_bf = spool.tile([1, NSEQ], bf16, name=f"r_bf_{nm}")
            for nb in range(NB4):
                sl = slice(nb * 512, (nb + 1) * 512)
                sq = sqpool.tile([128, KT, 512], bf16)
                nc.vector.tensor_mul(sq, xsb[:, :, sl], xsb[:, :, sl])
                mu_ps = pst.tile([1, 512], f32, name="mu_ps")
                ms_ps = pst.tile([1, 512], f32, name="ms_ps")
                for kt in range(KT):
                    nc.tensor.matmul(mu_ps, ones, xsb[:, kt, sl],
                                     start=(kt == 0), stop=(kt == KT - 1))
                for kt in range(KT):
                    nc.tensor.matmul(ms_ps, ones, sq[:, kt, :],
                                     start=(kt == 0), stop=(kt == KT - 1))
                mu5 = scr.tile([1, 512], f32, name="mu5")
                var5 = scr.tile([1, 512], f32, name="var5")
                mu25 = scr.tile([1, 512], f32, name="mu25")
                nc.vector.tensor_scalar_mul(mu5, mu_ps, 1.0 / D)
                nc.vector.tensor_scalar_mul(var5, ms_ps, 1.0 / D)
                nc.vector.tensor_mul(mu25, mu5, mu5)
                nc.vector.tensor_sub(var5, var5, mu25)
                # var -> sd -> rstd
                nc.scalar.activation(var5, var5, AF.Sqrt, bias=eps1)
                nc.vector.reciprocal(var5, var5)
                nc.vector.tensor_copy(mu_bf[:, sl], mu5)
                nc.vector.tensor_copy(r_bf[:, sl], var5)
            # broadcast rows across partitions (bf16)
            mu_bc = bcpool.tile([128, NSEQ], bf16, name=f"mu_bc_{nm}")
            r_bc = bcpool.tile([128, NSEQ], bf16, name=f"r_bc_{nm}")
            nc.gpsimd.partition_broadcast(mu_bc, mu_bf)
            nc.gpsimd.partition_broadcast(r_bc, r_bf)
            bcs[nm] = (mu_bc, r_bc)

        # ---- kv per-partition stats (for seq-major V fixup) via bn_stats ----
        mu_pp = ppool.tile([128, NB16], f32)
        r_pp = ppool.tile([128, NB16], f32)
        bn_sub = math.gcd(nc.vector.BN_STATS_FMAX, D)   # 256
        nsub = D // bn_sub
        for o in range(NB16):
            xt = xsm_pool.tile([128, D], bf16)
            nc.sync.dma_start(out=xt, in_=xkv_sm[o * 128:(o + 1) * 128, :])
            stats = bnp.tile([128, nsub, nc.vector.BN_STATS_DIM], f32)
            xr = xt.rearrange("p (s d) -> p s d", s=nsub)
            for si in range(nsub):
                nc.vector.bn_stats(out=stats[:, si, :], in_=xr[:, si, :])
            mv = bnp.tile([128, nc.vector.BN_AGGR_DIM], f32)
            nc.vector.bn_aggr(out=mv, in_=stats)
            nc.gpsimd.tensor_copy(out=mu_pp[:, o:o + 1], in_=mv[:, 0:1])
            nc.gpsimd.tensor_copy(out=r_pp[:, o:o + 1], in_=mv[:, 1:2])
        nc.scalar.activation(r_pp, r_pp, AF.Sqrt, bias=eps128)
        nc.vector.reciprocal(r_pp, r_pp)

        # ---- Q^T / K^T projections (feature-major) + LN fixup ----
        for xsb, wsb, cn_sb, dst, nm in (
            (xqT_sb, wq_sb, cnq_sb, qt_sb, "q"),
            (xkvT_sb, wk_sb, cnk_sb, kt_sb, "kv"),
        ):
            mu_bc, r_bc = bcs[nm]
            for fb in range(FB):
                pss = [prj_ps.tile([128, 512], f32, name=f"prj{i}") for i in range(NB4)]
                for kt in range(KT):
                    for nb in range(NB4):
                        nc.tensor.matmul(
                            pss[nb],
                            wsb[:, kt, fb * 128:(fb + 1) * 128],
                            xsb[:, kt, nb * 512:(nb + 1) * 512],
                            start=(kt == 0), stop=(kt == KT - 1))
                for nb in range(NB4):
                    sl = slice(nb * 512, (nb + 1) * 512)
                    u = upool.tile([128, 512], f32, name="u")
                    # u = raw - colsum * mu   (cn_sb holds -colsum)
                    nc.vector.scalar_tensor_tensor(
                        out=u, in0=mu_bc[:, sl], scalar=cn_sb[:, fb:fb + 1],
                        in1=pss[nb], op0=OP.mult, op1=OP.add)
                    nc.vector.tensor_mul(dst[:, fb, sl], u, r_bc[:, sl])

        # ---- V projection (seq-major) + LN fixup + bias ----
        for o in range(NB16):
            ps = prj_ps.tile([128, F], f32, name="vps")
            for kt in range(KT):
                nc.tensor.matmul(ps, xkvT_sb[:, kt, o * 128:(o + 1) * 128],
                                 wv_sb[:, kt, :], start=(kt == 0), stop=(kt == KT - 1))
            uv = upool.tile([128, F], f32, name="uv")
            nc.vector.scalar_tensor_tensor(
                out=uv, in0=cnv_bc, scalar=mu_pp[:, o:o + 1], in1=ps,
                op0=OP.mult, op1=OP.add)
            nc.vector.scalar_tensor_tensor(
                out=v_sb[:, o, :, 0:DK],
                in0=uv.rearrange("p (h d) -> p h d", h=HPC),
                scalar=r_pp[:, o:o + 1],
                in1=bv_bc.rearrange("p (h d) -> p h d", h=HPC),
                op0=OP.mult, op1=OP.add)
        nc.vector.memset(v_sb[:, :, :, DK:DK + 1], 1.0)

    # ================= phase B: attention =================
    attn = ctx.enter_context(ExitStack())
    att_ps = attn.enter_context(tc.tile_pool(name="att", bufs=3, space="PSUM"))
    o_ps_pool = attn.enter_context(tc.tile_pool(name="ops", bufs=1, space="PSUM"))
    t_ps_pool = attn.enter_context(tc.tile_pool(name="tps", bufs=1, space="PSUM"))
    ptpool = attn.enter_context(tc.tile_pool(name="pt", bufs=8))
    rspool = attn.enter_context(tc.tile_pool(name="rs", bufs=6))

    AFexp = AF.Exp
    for h in range(HPC):
        fb, half = h // 2, (h % 2) * 64
        # t_k = bq . K_k  (per-key exp bias), scaled by SCALE on evict
        tps = t_ps_pool.tile([128, NB16], f32, name="tps")
        for kb in range(NB16):
            nc.tensor.matmul(tps[:, kb:kb + 1],
                             kt_sb[half:half + 64, fb, kb * 128:(kb + 1) * 128],
                             bqc_sb[half:half + 64, fb:fb + 1],
                             start=True, stop=True)
        nc.vector.tensor_scalar_mul(t_sb[:, h, :], tps, SCALE)

        opss = [o_ps_pool.tile([DK + 1, 512], f32, name=f"o{qb}") for qb in range(NB4)]
        for kb in range(NB16):
            ksl = kt_sb[half:half + 64, fb, kb * 128:(kb + 1) * 128]
            pts = []
            for qb in range(NB4):
                sps = att_ps.tile([128, 512], f32, name="sps")
                nc.tensor.matmul(sps, ksl,
                                 qt_sb[half:half + 64, fb, qb * 512:(qb + 1) * 512],
                                 start=True, stop=True)
                pt = ptpool.tile([128, 512], bf16, name="pt")
                nc.scalar.activation(pt, sps, AFexp,
                                     bias=t_sb[:, h, kb:kb + 1], scale=SCALE)
                pts.append(pt)
            vsl = v_sb[:, kb, h, :]   # [128, 65]
            for qb in range(NB4):
                nc.tensor.matmul(opss[qb], vsl, pts[qb],
                                 start=(kb == 0), stop=(kb == NB16 - 1))
        for qb in range(NB4):
            rs_row = rspool.tile([1, 512], f32, name="rsrow")
            nc.vector.reciprocal(rs_row, opss[qb][DK:DK + 1, :])
            rs_bc = rspool.tile([64, 512], f32, name="rsbc")
            nc.gpsimd.partition_broadcast(rs_bc, rs_row)
            nc.vector.tensor_mul(
                a_sb[half:half + 64, fb, qb * 512:(qb + 1) * 512],
                opss[qb][0:DK, :], rs_bc)

    # ================= phase C: output projection =================
    attn.close()
    op_ps = ctx.enter_context(tc.tile_pool(name="oprj", bufs=2, space="PSUM"))
    outpool = ctx.enter_context(tc.tile_pool(name="outsb", bufs=3))
    for mb in range(NB16):
        pss = [op_ps.tile([128, 384], f32, name=f"op{j}") for j in range(2)]
        for kt3 in range(FB):
            asl = a_sb[:, kt3, mb * 128:(mb + 1) * 128]
            for j in range(2):
                nc.tensor.matmul(pss[j], asl, wo_sb[:, kt3, j * 384:(j + 1) * 384],
                                 start=(kt3 == 0), stop=(kt3 == FB - 1))
        osb = outpool.tile([128, D], f32)
        for j in range(2):
            nc.vector.tensor_copy(osb[:, j * 384:(j + 1) * 384], pss[j])
        nc.sync.dma_start(out=out[mb * 128:(mb + 1) * 128, :], in_=osb)


def _build():
    nc = bacc.Bacc("TRN2", target_bir_lowering=False, debug=False, num_devices=8)
    dt = mybir.dt

    def din(name, shape, dtype):
        return nc.dram_tensor(name, list(shape), dtype, kind="ExternalInput").ap()

    io = {
        "xqT": din("xqT", (D, NSEQ), dt.float16),
        "xkvT": din("xkvT", (D, NSEQ), dt.float16),
        "xkv_sm": din("xkv_sm", (NSEQ, D), dt.float16),
        "wq": din("wq", (KT, 128, F), dt.float16),
        "wk": din("wk", (KT, 128, F), dt.float16),
        "wv": din("wv", (KT, 128, F), dt.float16),
        "wo": din("wo", (FB, 128, D), dt.float16),
        "cnq": din("cnq", (128, FB), dt.float32),
        "cnk": din("cnk", (128, FB), dt.float32),
        "cnv": din("cnv", (F,), dt.float32),
        "bvr": din("bvr", (F,), dt.float32),
        "bqc": din("bqc", (128, FB), dt.float16),
        "out": nc.dram_tensor("out", [NSEQ, D], dt.float32, kind="ExternalOutput").ap(),
    }

    with tile.TileContext(nc) as tc:
        with ExitStack() as ctx:
            _emit(ctx, tc, io)
    nc.compile()
    return nc


_CACHE = {}


def _get_nc():
    if "nc" not in _CACHE:
        _CACHE["nc"] = _build()
    return _CACHE["nc"]


def _prep(inputs):
    g = lambda k: np.asarray(inputs[k], dtype=np.float32)
    text, vision = g("text"), g("vision")
    ln1_w, ln1_b, ln2_w, ln2_b = g("ln1_w"), g("ln1_b"), g("ln2_w"), g("ln2_b")
    W = {nm: g("W" + nm) for nm in ("q1", "k1", "v1", "q2", "k2", "v2", "o1", "o2")}
    B = {nm: g("b" + nm) for nm in ("q1", "k1", "v1", "q2", "k2", "v2", "o1", "o2")}

    maps = [None] * 8
    for b in (0, 1):
        for path in (0, 1):
            if path == 0:
                xq, xkv = text[b], vision[b]
                lnqw, lnqb, lnkw, lnkb = ln1_w, ln1_b, ln2_w, ln2_b
                Wq, bq, Wk, Wv, bv, Wo = W["q1"], B["q1"], W["k2"], W["v2"], B["v2"], W["o1"]
            else:
                xq, xkv = vision[b], text[b]
                lnqw, lnqb, lnkw, lnkb = ln2_w, ln2_b, ln1_w, ln1_b
                Wq, bq, Wk, Wv, bv, Wo = W["q2"], B["q2"], W["k1"], W["v1"], B["v1"], W["o2"]
            xqT = np.ascontiguousarray(xq.T).astype(BF16)
            xkvT = np.ascontiguousarray(xkv.T).astype(BF16)
            xkv_sm = xkv.astype(BF16)
            for s in (0, 1):
                rows = slice(s * F, (s + 1) * F)
                WqT = np.ascontiguousarray((lnqw[:, None] * Wq[rows].T)).astype(BF16)
                WkT = np.ascontiguousarray((lnkw[:, None] * Wk[rows].T)).astype(BF16)
                WvT = np.ascontiguousarray((lnkw[:, None] * Wv[rows].T)).astype(BF16)
                cq = -WqT.astype(np.float32).sum(0)   # [F]
                ck = -WkT.astype(np.float32).sum(0)
                cv = -WvT.astype(np.float32).sum(0)
                bq_eff = (bq[rows] + lnqb @ Wq[rows].T).astype(np.float32)
                bv_eff = (bv[rows] + lnkb @ Wv[rows].T).astype(np.float32)
                WoT = np.ascontiguousarray(Wo[:, rows].T).astype(BF16)  # [F, D]
                maps[b * 4 + path * 2 + s] = {
                    "xqT": xqT, "xkvT": xkvT, "xkv_sm": xkv_sm,
                    "wq": WqT.reshape(KT, 128, F),
                    "wk": WkT.reshape(KT, 128, F),
                    "wv": WvT.reshape(KT, 128, F),
                    "wo": WoT.reshape(FB, 128, D),
                    "cnq": np.ascontiguousarray(cq.reshape(FB, 128).T),
                    "cnk": np.ascontiguousarray(ck.reshape(FB, 128).T),
                    "cnv": cv,
                    "bvr": bv_eff,
                    "bqc": np.ascontiguousarray(bq_eff.reshape(FB, 128).T).astype(BF16),
                }
    meta = (B["o1"], B["o2"])
    return maps, meta


def _unshard(results, meta):
    bo1, bo2 = meta
    text_out = np.empty((2, NSEQ, D), np.float32)
    vision_out = np.empty((2, NSEQ, D), np.float32)
    for b in (0, 1):
        text_out[b] = results[b * 4 + 0]["out"] + results[b * 4 + 1]["out"] + bo1
        vision_out[b] = results[b * 4 + 2]["out"] + results[b * 4 + 3]["out"] + bo2
    return (text_out, vision_out)


def run_raw(inputs, **kw):
    """Run and return the BassKernelResults (for profiling from test.py)."""
    nc = _get_nc()
    in_maps, meta = _prep(inputs)
    res = run_bass_kernel_spmd(nc, in_maps, core_ids=list(range(8)), **kw)
    return res, meta


def kernel(**inputs):
    res, meta = run_raw(inputs)
    return _unshard(res.results, meta)



# revision 20
# speedup vs baseline: 1.0620x; 1.0620x over previous
"""Trainium2 Bass kernel for a bidirectional cross-attention block.

Reference computation (per batch b):
  t = LN(text[b]);  v = LN(vision[b])
  text_out[b]   = softmax((t@Wq1.T+bq1) (v@Wk2.T+bk2)^T / 8) (v@Wv2.T+bv2) @ Wo1.T + bo1
  vision_out[b] = softmax((v@Wq2.T+bq2) (t@Wk1.T+bk1)^T / 8) (t@Wv1.T+bv1) @ Wo2.T + bo2
  (12 heads of dk=64; D=768, N=2048)

Sharding over 8 cores: (batch b in {0,1}) x (path in {text-q, vision-q}) x
(head-half in {heads 0-5, heads 6-11}).  Each core computes a [2048, 768]
partial of one output (its 6 heads pushed through the output projection);
the host sums the two head-half partials and adds the output bias.

Device kernel (per core) highlights:
  - Host passes x TRANSPOSED (feature-major, bf16) so every matmul contracts
    along partitions; activations are never transposed on device.
  - LN scale/shift folded into the projection weights host-side (exact).
    LN mean/rstd computed on device:  Q = r*(x@W) - (r*mu) x colsum(W),
    applied as two DVE passes per projection tile (colsum identity).
  - K-side biases are row-constant in the scores -> dropped (softmax
    invariant).  The Q bias enters through a per-key correction t_k =
    bq . K_k, computed with tiny matmuls and folded into the exp() bias.
  - exp on ACT with fused scale 1/8 + bias; no max-subtraction (scores are
    bounded by ~6 for LN'd inputs).
  - Row-sums of exp(S) obtained by appending a ones-column to V in the
    P@V matmul; normalization applied after P@V (linearity).
"""

import math
import os
import sys
from contextlib import ExitStack

import numpy as np

for _p in ("/opt/trn_rl_repo", os.path.expanduser("~/.axon_site/_ro/trn_rl_repo")):
    if os.path.isdir(_p) and _p not in sys.path:
        sys.path.insert(0, _p)

import ml_dtypes  # noqa: E402

import concourse.bass as bass  # noqa: E402
import concourse.bacc as bacc  # noqa: E402
import concourse.tile as tile  # noqa: E402
from concourse import mybir  # noqa: E402
from concourse.bass_utils import run_bass_kernel_spmd  # noqa: E402

BF16 = np.dtype(np.float16)  # fp16: same PE rate as bf16, 8x mantissa

NSEQ = 2048
D = 768
HEADS = 12
DK = 64
HPC = 6            # heads per core
F = HPC * DK       # 384 features per core
KT = D // 128      # 6 contraction tiles
FB = F // 128      # 3 feature blocks
NB4 = NSEQ // 512  # 4 seq blocks of 512
NB16 = NSEQ // 128  # 16 seq blocks of 128
EPS = 1e-5
SCALE = DK ** -0.5  # 0.125

_AF = None  # mybir.ActivationFunctionType alias, set in _build


def _bcast_ap(ap, p):
    """[1, ...] SBUF/DRAM AP -> partition-broadcast [p, ...] AP (stride 0)."""
    return bass.AP(tensor=ap.tensor, offset=ap.offset, ap=[[0, p]] + list(ap.ap[1:]))


def _emit(ctx, tc, io):
    nc = tc.nc
    f32 = mybir.dt.float32
    bf16 = mybir.dt.float16
    AF = mybir.ActivationFunctionType
    OP = mybir.AluOpType

    xqT, xkvT, xkv_sm = io["xqT"], io["xkvT"], io["xkv_sm"]
    wq, wk, wv, wo = io["wq"], io["wk"], io["wv"], io["wo"]
    cnq, cnk, cnv, bvr, bqc = io["cnq"], io["cnk"], io["cnv"], io["bvr"], io["bqc"]
    out = io["out"]

    # ---- pools that live for the whole kernel ----
    const = ctx.enter_context(tc.tile_pool(name="const", bufs=1))
    qkv_pool = ctx.enter_context(tc.tile_pool(name="qkv", bufs=1))
    wo_pool = ctx.enter_context(tc.tile_pool(name="wo", bufs=1))
    tpool = ctx.enter_context(tc.tile_pool(name="tsb", bufs=1))
    apool = ctx.enter_context(tc.tile_pool(name="afm", bufs=1))

    ones = const.tile([128, 1], bf16)
    nc.vector.memset(ones, 1.0)
    eps1 = const.tile([1, 1], f32)
    nc.vector.memset(eps1, EPS)
    eps128 = const.tile([128, 1], f32)
    nc.vector.memset(eps128, EPS)
    cnq_sb = const.tile([128, FB], f32)
    nc.sync.dma_start(out=cnq_sb, in_=cnq)
    cnk_sb = const.tile([128, FB], f32)
    nc.sync.dma_start(out=cnk_sb, in_=cnk)
    bqc_sb = const.tile([128, FB], bf16)
    nc.sync.dma_start(out=bqc_sb, in_=bqc)
    cnv_bc = const.tile([128, F], f32)
    nc.gpsimd.dma_start(out=cnv_bc, in_=_bcast_ap(cnv[None, :], 128))
    bv_bc = const.tile([128, F], f32)
    nc.gpsimd.dma_start(out=bv_bc, in_=_bcast_ap(bvr[None, :], 128))

    qt_sb = qkv_pool.tile([128, FB, NSEQ], bf16)   # Q^T feature-major
    kt_sb = qkv_pool.tile([128, FB, NSEQ], bf16)   # K^T feature-major
    v_sb = qkv_pool.tile([128, NB16, HPC, DK + 1], bf16)  # V seq-major + ones col
    a_sb = apool.tile([128, FB, NSEQ], bf16)       # attention out, feature-major

    wo_sb = wo_pool.tile([128, FB, D], bf16)
    for f3 in range(FB):
        nc.sync.dma_start(out=wo_sb[:, f3, :], in_=wo[f3])

    t_sb = tpool.tile([128, HPC, NB16], f32)       # per-key exp-bias (q-bias fold)

    # ================= phase A: load x, stats, projections =================
    with ExitStack() as pre:
        xpool = pre.enter_context(tc.tile_pool(name="xt", bufs=1))
        wpool = pre.enter_context(tc.tile_pool(name="wqkv", bufs=1))
        spool = pre.enter_context(tc.tile_pool(name="stats", bufs=1))
        scr = pre.enter_context(tc.tile_pool(name="scratch", bufs=2))
        sqpool = pre.enter_context(tc.tile_pool(name="sq", bufs=2))
        bcpool = pre.enter_context(tc.tile_pool(name="bc", bufs=1))
        ppool = pre.enter_context(tc.tile_pool(name="pp", bufs=1))
        xsm_pool = pre.enter_context(tc.tile_pool(name="xsm", bufs=3))
        bnp = pre.enter_context(tc.tile_pool(name="bn", bufs=3))
        upool = pre.enter_context(tc.tile_pool(name="u", bufs=3))
        pst = pre.enter_context(tc.tile_pool(name="pstat", bufs=1, space="PSUM"))
        prj_ps = pre.enter_context(tc.tile_pool(name="prj", bufs=1, space="PSUM"))

        xqT_sb = xpool.tile([128, KT, NSEQ], bf16)
        xkvT_sb = xpool.tile([128, KT, NSEQ], bf16)
        for kt in range(KT):
            nc.sync.dma_start(out=xqT_sb[:, kt, :], in_=xqT[kt * 128:(kt + 1) * 128, :])
            nc.sync.dma_start(out=xkvT_sb[:, kt, :], in_=xkvT[kt * 128:(kt + 1) * 128, :])

        wq_sb = wpool.tile([128, KT, F], bf16)
        wk_sb = wpool.tile([128, KT, F], bf16)
        wv_sb = wpool.tile([128, KT, F], bf16)
        for kt in range(KT):
            nc.sync.dma_start(out=wq_sb[:, kt, :], in_=wq[kt])
            nc.sync.dma_start(out=wk_sb[:, kt, :], in_=wk[kt])
            nc.sync.dma_start(out=wv_sb[:, kt, :], in_=wv[kt])

        # ---- row stats (mean, rstd along features) via ones-matmuls ----
        bcs = {}
        for nm, xsb in (("q", xqT_sb), ("kv", xkvT_sb)):
            mu_bf = spool.tile([1, NSEQ], bf16, name=f"mu_bf_{nm}")
            r_bf = spool.tile([1, NSEQ], bf16, name=f"r_bf_{nm}")
            for nb in range(NB4):
                sl = slice(nb * 512, (nb + 1) * 512)
                sq = sqpool.tile([128, KT, 512], bf16)
                nc.vector.tensor_mul(sq, xsb[:, :, sl], xsb[:, :, sl])
                mu_ps = pst.tile([1, 512], f32, name="mu_ps")
                ms_ps = pst.tile([1, 512], f32, name="ms_ps")
                for kt in range(KT):
                    nc.tensor.matmul(mu_ps, ones, xsb[:, kt, sl],
                                     start=(kt == 0), stop=(kt == KT - 1))
                for kt in range(KT):
                    nc.tensor.matmul(ms_ps, ones, sq[:, kt, :],
                                     start=(kt == 0), stop=(kt == KT - 1))
                mu5 = scr.tile([1, 512], f32, name="mu5")
                var5 = scr.tile([1, 512], f32, name="var5")
                mu25 = scr.tile([1, 512], f32, name="mu25")
                nc.vector.tensor_scalar_mul(mu5, mu_ps, 1.0 / D)
                nc.vector.tensor_scalar_mul(var5, ms_ps, 1.0 / D)
                nc.vector.tensor_mul(mu25, mu5, mu5)
                nc.vector.tensor_sub(var5, var5, mu25)
                # var -> sd -> rstd
                nc.scalar.activation(var5, var5, AF.Sqrt, bias=eps1)
                nc.vector.reciprocal(var5, var5)
                nc.vector.tensor_copy(mu_bf[:, sl], mu5)
                nc.vector.tensor_copy(r_bf[:, sl], var5)
            # broadcast rows across partitions (bf16)
            mu_bc = bcpool.tile([128, NSEQ], bf16, name=f"mu_bc_{nm}")
            r_bc = bcpool.tile([128, NSEQ], bf16, name=f"r_bc_{nm}")
            nc.gpsimd.partition_broadcast(mu_bc, mu_bf)
            nc.gpsimd.partition_broadcast(r_bc, r_bf)
            bcs[nm] = (mu_bc, r_bc)

        # ---- kv per-partition stats (for seq-major V fixup) via bn_stats ----
        mu_pp = ppool.tile([128, NB16], f32)
        r_pp = ppool.tile([128, NB16], f32)
        bn_sub = math.gcd(nc.vector.BN_STATS_FMAX, D)   # 256
        nsub = D // bn_sub
        for o in range(NB16):
            xt = xsm_pool.tile([128, D], bf16)
            nc.sync.dma_start(out=xt, in_=xkv_sm[o * 128:(o + 1) * 128, :])
            stats = bnp.tile([128, nsub, nc.vector.BN_STATS_DIM], f32)
            xr = xt.rearrange("p (s d) -> p s d", s=nsub)
            for si in range(nsub):
                nc.vector.bn_stats(out=stats[:, si, :], in_=xr[:, si, :])
            mv = bnp.tile([128, nc.vector.BN_AGGR_DIM], f32)
            nc.vector.bn_aggr(out=mv, in_=stats)
            nc.gpsimd.tensor_copy(out=mu_pp[:, o:o + 1], in_=mv[:, 0:1])
            nc.gpsimd.tensor_copy(out=r_pp[:, o:o + 1], in_=mv[:, 1:2])
        nc.scalar.activation(r_pp, r_pp, AF.Sqrt, bias=eps128)
        nc.vector.reciprocal(r_pp, r_pp)

        # ---- Q^T / K^T projections (feature-major) + LN fixup ----
        for xsb, wsb, cn_sb, dst, nm in (
            (xqT_sb, wq_sb, cnq_sb, qt_sb, "q"),
            (xkvT_sb, wk_sb, cnk_sb, kt_sb, "kv"),
        ):
            mu_bc, r_bc = bcs[nm]
            for fb in range(FB):
                pss = [prj_ps.tile([128, 512], f32, name=f"prj{i}") for i in range(NB4)]
                for kt in range(KT):
                    for nb in range(NB4):
                        nc.tensor.matmul(
                            pss[nb],
                            wsb[:, kt, fb * 128:(fb + 1) * 128],
                            xsb[:, kt, nb * 512:(nb + 1) * 512],
                            start=(kt == 0), stop=(kt == KT - 1))
                for nb in range(NB4):
                    sl = slice(nb * 512, (nb + 1) * 512)
                    u = upool.tile([128, 512], f32, name="u")
                    # u = raw - colsum * mu   (cn_sb holds -colsum)
                    nc.vector.scalar_tensor_tensor(
                        out=u, in0=mu_bc[:, sl], scalar=cn_sb[:, fb:fb + 1],
                        in1=pss[nb], op0=OP.mult, op1=OP.add)
                    nc.vector.tensor_mul(dst[:, fb, sl], u, r_bc[:, sl])

        # ---- V projection (seq-major) + LN fixup + bias ----
        for o in range(NB16):
            ps = prj_ps.tile([128, F], f32, name="vps")
            for kt in range(KT):
                nc.tensor.matmul(ps, xkvT_sb[:, kt, o * 128:(o + 1) * 128],
                                 wv_sb[:, kt, :], start=(kt == 0), stop=(kt == KT - 1))
            uv = upool.tile([128, F], f32, name="uv")
            nc.vector.scalar_tensor_tensor(
                out=uv, in0=cnv_bc, scalar=mu_pp[:, o:o + 1], in1=ps,
                op0=OP.mult, op1=OP.add)
            nc.vector.scalar_tensor_tensor(
                out=v_sb[:, o, :, 0:DK],
                in0=uv.rearrange("p (h d) -> p h d", h=HPC),
                scalar=r_pp[:, o:o + 1],
                in1=bv_bc.rearrange("p (h d) -> p h d", h=HPC),
                op0=OP.mult, op1=OP.add)
        nc.vector.memset(v_sb[:, :, :, DK:DK + 1], 1.0)

        # per-key exp bias t_k = SCALE*(bq . K_k) for all heads (frees a
        # PSUM bank in phase B so score tiles can be 1024 wide)
        for h in range(HPC):
            fb, half = h // 2, (h % 2) * 64
            tps = pst.tile([128, NB16], f32, name="tps")
            for kb in range(NB16):
                nc.tensor.matmul(tps[:, kb:kb + 1],
                                 kt_sb[half:half + 64, fb, kb * 128:(kb + 1) * 128],
                                 bqc_sb[half:half + 64, fb:fb + 1],
                                 start=True, stop=True)
            nc.vector.tensor_scalar_mul(t_sb[:, h, :], tps, SCALE)

    # ================= phase B: attention =================
    attn = ctx.enter_context(ExitStack())
    att_ps = attn.enter_context(tc.tile_pool(name="att", bufs=2, space="PSUM"))
    o_ps_pool = attn.enter_context(tc.tile_pool(name="ops", bufs=1, space="PSUM"))
    ptpool = attn.enter_context(tc.tile_pool(name="pt", bufs=4))
    rspool = attn.enter_context(tc.tile_pool(name="rs", bufs=6))

    AFexp = AF.Exp
    for h in range(HPC):
        fb, half = h // 2, (h % 2) * 64
        opss = [o_ps_pool.tile([DK + 1, 512], f32, name=f"o{qb}") for qb in range(NB4)]
        for kb in range(NB16):
            ksl = kt_sb[half:half + 64, fb, kb * 128:(kb + 1) * 128]
            pts = []
            for t in range(2):
                sps = att_ps.tile([128, 1024], f32, name="sps")
                for g in range(2):
                    qb = 2 * t + g
                    nc.tensor.matmul(sps[:, g * 512:(g + 1) * 512], ksl,
                                     qt_sb[half:half + 64, fb, qb * 512:(qb + 1) * 512],
                                     start=True, stop=True)
                pt = ptpool.tile([128, 1024], bf16, name="pt")
                nc.scalar.activation(pt, sps, AFexp,
                                     bias=t_sb[:, h, kb:kb + 1], scale=SCALE)
                pts.append(pt)
            vsl = v_sb[:, kb, h, :]   # [128, 65]
            for qb in range(NB4):
                nc.tensor.matmul(opss[qb], vsl,
                                 pts[qb // 2][:, (qb % 2) * 512:(qb % 2 + 1) * 512],
                                 start=(kb == 0), stop=(kb == NB16 - 1))
        for qb in range(NB4):
            rs_row = rspool.tile([1, 512], f32, name="rsrow")
            nc.vector.reciprocal(rs_row, opss[qb][DK:DK + 1, :])
            rs_bc = rspool.tile([64, 512], f32, name="rsbc")
            nc.gpsimd.partition_broadcast(rs_bc, rs_row)
            nc.vector.tensor_mul(
                a_sb[half:half + 64, fb, qb * 512:(qb + 1) * 512],
                opss[qb][0:DK, :], rs_bc)

    # ================= phase C: output projection =================
    attn.close()
    op_ps = ctx.enter_context(tc.tile_pool(name="oprj", bufs=2, space="PSUM"))
    outpool = ctx.enter_context(tc.tile_pool(name="outsb", bufs=3))
    for mb in range(NB16):
        pss = [op_ps.tile([128, 384], f32, name=f"op{j}") for j in range(2)]
        for kt3 in range(FB):
            asl = a_sb[:, kt3, mb * 128:(mb + 1) * 128]
            for j in range(2):
                nc.tensor.matmul(pss[j], asl, wo_sb[:, kt3, j * 384:(j + 1) * 384],
                                 start=(kt3 == 0), stop=(kt3 == FB - 1))
        osb = outpool.tile([128, D], f32)
        for j in range(2):
            nc.vector.tensor_copy(osb[:, j * 384:(j + 1) * 384], pss[j])
        nc.sync.dma_start(out=out[mb * 128:(mb + 1) * 128, :], in_=osb)


def _build():
    nc = bacc.Bacc("TRN2", target_bir_lowering=False, debug=False, num_devices=8)
    dt = mybir.dt

    def din(name, shape, dtype):
        return nc.dram_tensor(name, list(shape), dtype, kind="ExternalInput").ap()

    io = {
        "xqT": din("xqT", (D, NSEQ), dt.float16),
        "xkvT": din("xkvT", (D, NSEQ), dt.float16),
        "xkv_sm": din("xkv_sm", (NSEQ, D), dt.float16),
        "wq": din("wq", (KT, 128, F), dt.float16),
        "wk": din("wk", (KT, 128, F), dt.float16),
        "wv": din("wv", (KT, 128, F), dt.float16),
        "wo": din("wo", (FB, 128, D), dt.float16),
        "cnq": din("cnq", (128, FB), dt.float32),
        "cnk": din("cnk", (128, FB), dt.float32),
        "cnv": din("cnv", (F,), dt.float32),
        "bvr": din("bvr", (F,), dt.float32),
        "bqc": din("bqc", (128, FB), dt.float16),
        "out": nc.dram_tensor("out", [NSEQ, D], dt.float32, kind="ExternalOutput").ap(),
    }

    with tile.TileContext(nc) as tc:
        with ExitStack() as ctx:
            _emit(ctx, tc, io)
    nc.compile()
    return nc


_CACHE = {}


def _get_nc():
    if "nc" not in _CACHE:
        _CACHE["nc"] = _build()
    return _CACHE["nc"]


def _prep(inputs):
    g = lambda k: np.asarray(inputs[k], dtype=np.float32)
    text, vision = g("text"), g("vision")
    ln1_w, ln1_b, ln2_w, ln2_b = g("ln1_w"), g("ln1_b"), g("ln2_w"), g("ln2_b")
    W = {nm: g("W" + nm) for nm in ("q1", "k1", "v1", "q2", "k2", "v2", "o1", "o2")}
    B = {nm: g("b" + nm) for nm in ("q1", "k1", "v1", "q2", "k2", "v2", "o1", "o2")}

    maps = [None] * 8
    for b in (0, 1):
        for path in (0, 1):
            if path == 0:
                xq, xkv = text[b], vision[b]
                lnqw, lnqb, lnkw, lnkb = ln1_w, ln1_b, ln2_w, ln2_b
                Wq, bq, Wk, Wv, bv, Wo = W["q1"], B["q1"], W["k2"], W["v2"], B["v2"], W["o1"]
            else:
                xq, xkv = vision[b], text[b]
                lnqw, lnqb, lnkw, lnkb = ln2_w, ln2_b, ln1_w, ln1_b
                Wq, bq, Wk, Wv, bv, Wo = W["q2"], B["q2"], W["k1"], W["v1"], B["v1"], W["o2"]
            xqT = np.ascontiguousarray(xq.T).astype(BF16)
            xkvT = np.ascontiguousarray(xkv.T).astype(BF16)
            xkv_sm = xkv.astype(BF16)
            for s in (0, 1):
                rows = slice(s * F, (s + 1) * F)
                WqT = np.ascontiguousarray((lnqw[:, None] * Wq[rows].T)).astype(BF16)
                WkT = np.ascontiguousarray((lnkw[:, None] * Wk[rows].T)).astype(BF16)
                WvT = np.ascontiguousarray((lnkw[:, None] * Wv[rows].T)).astype(BF16)
                cq = -WqT.astype(np.float32).sum(0)   # [F]
                ck = -WkT.astype(np.float32).sum(0)
                cv = -WvT.astype(np.float32).sum(0)
                bq_eff = (bq[rows] + lnqb @ Wq[rows].T).astype(np.float32)
                bv_eff = (bv[rows] + lnkb @ Wv[rows].T).astype(np.float32)
                WoT = np.ascontiguousarray(Wo[:, rows].T).astype(BF16)  # [F, D]
                maps[b * 4 + path * 2 + s] = {
                    "xqT": xqT, "xkvT": xkvT, "xkv_sm": xkv_sm,
                    "wq": WqT.reshape(KT, 128, F),
                    "wk": WkT.reshape(KT, 128, F),
                    "wv": WvT.reshape(KT, 128, F),
                    "wo": WoT.reshape(FB, 128, D),
                    "cnq": np.ascontiguousarray(cq.reshape(FB, 128).T),
                    "cnk": np.ascontiguousarray(ck.reshape(FB, 128).T),
                    "cnv": cv,
                    "bvr": bv_eff,
                    "bqc": np.ascontiguousarray(bq_eff.reshape(FB, 128).T).astype(BF16),
                }
    meta = (B["o1"], B["o2"])
    return maps, meta


def _unshard(results, meta):
    bo1, bo2 = meta
    text_out = np.empty((2, NSEQ, D), np.float32)
    vision_out = np.empty((2, NSEQ, D), np.float32)
    for b in (0, 1):
        text_out[b] = results[b * 4 + 0]["out"] + results[b * 4 + 1]["out"] + bo1
        vision_out[b] = results[b * 4 + 2]["out"] + results[b * 4 + 3]["out"] + bo2
    return (text_out, vision_out)


def run_raw(inputs, **kw):
    """Run and return the BassKernelResults (for profiling from test.py)."""
    nc = _get_nc()
    in_maps, meta = _prep(inputs)
    res = run_bass_kernel_spmd(nc, in_maps, core_ids=list(range(8)), **kw)
    return res, meta


def kernel(**inputs):
    res, meta = run_raw(inputs)
    return _unshard(res.results, meta)

